# revision 1
# baseline (speedup 1.0000x reference)
"""DglGraphConvolution Trainium2 kernel — pure-matmul aggregation.

Per graph:
  1. PE: hidden = text @ W; kept in SBUF as bf16 [128, 32, 130]
     (32 windows of 128 node rows; col 128 = 1.0 degree lane, col 129 = 0).
  2. Edges sorted by (dst_window, src_window) into 32x32 blocks of the
     128x128 node grid; each block padded to exactly T_B=2 tiles of 128
     edge lanes (pad lanes have all-zero one-hot columns).
  3. For tile t (block b = t//2, ws = b % 32, wd = b // 32):
       mm1: gath_ps[128 lane, 130] = srcOH_t^T @ hidsb[:, ws, :]
            (lhsT = srcOH [128 src-node, 128 lane] bf16, shipped)
       copy: gath_sb bf16 <- gath_ps  (DVE/ACT)
       mm2: agg_ps[128 node, 130] += dstOH_t^T @ gath_sb
            (lhsT = dstOH [128 lane, 128 dst-node] bf16, shipped)
     agg_ps accumulates over the 64 tiles of each dst window; col 128 = deg.
  4. DVE: out = agg * 1/(deg+1) + bias per window.

Everything on device uses only plain DMA loads, matmuls, and elementwise
ops. Host-side work: sharding plus a bijective re-encoding of the edge
index lists into one-hot tiles (no arithmetic touches model data).
"""

import numpy as np

B, N, E, F = 16, 4096, 131072, 128
NCORES = 8
GPC = B // NCORES  # graphs per core
W = 128  # window size
NW = N // W  # 32
T_B = 2  # tiles per (wd, ws) block; Poisson(128) <= 256 w.p. ~1
T = NW * NW * T_B  # 2048 tiles per graph
HC = 130  # feature row: 128 | ones | pad
TPC = 64  # tiles per OH load chunk (= one dst window)
NCHUNK = T // TPC  # 32

_cache = {}


def _build_program():
    from contextlib import ExitStack

    import concourse.bacc as bacc
    import concourse.tile as tile
    from concourse import mybir
    from concourse._compat import get_trn_type
    from concourse.masks import make_identity

    f32 = mybir.dt.float32
    bf16 = mybir.dt.bfloat16

    nc = bacc.Bacc(get_trn_type() or "TRN2", target_bir_lowering=False, debug=False)

    text_d = nc.dram_tensor("text", [GPC, N, F], f32, kind="ExternalInput")
    w_d = nc.dram_tensor("weight", [F, F], f32, kind="ExternalInput")
    bias_d = nc.dram_tensor("biasrep", [128, F], f32, kind="ExternalInput")
    # pre-swizzled one-hots: [graph, chunk, lane/node, tile-in-chunk, 128]
    soh_d = nc.dram_tensor(
        "srcoh", [GPC, NCHUNK, 128, TPC, 128], bf16, kind="ExternalInput"
    )
    doh_d = nc.dram_tensor(
        "dstoh", [GPC, NCHUNK, 128, TPC, 128], bf16, kind="ExternalInput"
    )
    out_d = nc.dram_tensor("out", [GPC, N, F], f32, kind="ExternalOutput")

    with tile.TileContext(nc) as tc, ExitStack() as ctx:
        const = ctx.enter_context(tc.tile_pool(name="const", bufs=1))
        work = ctx.enter_context(tc.tile_pool(name="work", bufs=3))
        hpool = ctx.enter_context(tc.tile_pool(name="hpool", bufs=1))
        ohpool = ctx.enter_context(tc.tile_pool(name="ohp", bufs=2))
        gsb = ctx.enter_context(tc.tile_pool(name="gsb", bufs=4))
        psum = ctx.enter_context(tc.tile_pool(name="psum", bufs=1, space="PSUM"))
        gpsum = ctx.enter_context(tc.tile_pool(name="gpsum", bufs=3, space="PSUM"))
        apsum = ctx.enter_context(tc.tile_pool(name="apsum", bufs=1, space="PSUM"))

        ident = const.tile([128, 128], f32)
        make_identity(nc, ident[:])
        w_sb = const.tile([128, F], f32)
        nc.sync.dma_start(w_sb[:], w_d[:, :])
        bias_sb = const.tile([128, F], f32)
        nc.sync.dma_start(bias_sb[:], bias_d[:, :])

        agg_tiles = {}
        for g in range(GPC):
            # hidden = text @ W -> SBUF bf16 [128, 32, 130]
            hidsb = hpool.tile([128, NW, HC], bf16, tag="hidsb")
            nc.vector.memset(hidsb[:], 1.0)
            for c in range(NW):
                ttile = work.tile([128, F], f32, tag="text")
                nc.sync.dma_start(ttile[:], text_d[g, 128 * c : 128 * (c + 1), :])
                tT_ps = psum.tile([128, 128], f32, tag="tT")
                nc.tensor.transpose(out=tT_ps[:], in_=ttile[:], identity=ident[:])
                tT_sb = work.tile([128, 128], f32, tag="tTs")
                nc.vector.tensor_copy(tT_sb[:], tT_ps[:])
                h_ps = psum.tile([128, F], f32, tag="h")
                nc.tensor.matmul(
                    out=h_ps[:], lhsT=tT_sb[:], rhs=w_sb[:], start=True, stop=True
                )
                nc.scalar.activation(
                    hidsb[:, c, 0:F], h_ps[:], mybir.ActivationFunctionType.Copy
                )
                nc.vector.memset(hidsb[:, c, F + 1 : HC], 0.0)

            for chunk in range(NCHUNK):
                soh = ohpool.tile([128, TPC, 128], bf16, tag="soh")
                nc.sync.dma_start(soh[:], soh_d[g, chunk])
                doh = ohpool.tile([128, TPC, 128], bf16, tag="doh")
                nc.sync.dma_start(doh[:], doh_d[g, chunk])
                for tt in range(TPC):
                    t = chunk * TPC + tt
                    blk = t // T_B
                    ws = blk % NW
                    wd = blk // NW
                    j = t % TPC  # position within the dst window (== tt)
                    gath_ps = gpsum.tile([128, HC], f32, tag="gps")
                    nc.tensor.matmul(
                        out=gath_ps[:],
                        lhsT=soh[:, tt, :],
                        rhs=hidsb[:, ws, :],
                        start=True,
                        stop=True,
                    )
                    gath_sb = gsb.tile([128, HC], bf16, tag="gsb")
                    nc.vector.tensor_copy(gath_sb[:], gath_ps[:])
                    if j == 0:
                        agg_new = apsum.tile([128, HC], f32, tag=f"agg{wd % 2}")
                        agg_tiles[wd % 2] = agg_new
                    agg_ps = agg_tiles[wd % 2]
                    nc.tensor.matmul(
                        out=agg_ps[:],
                        lhsT=doh[:, tt, :],
                        rhs=gath_sb[:],
                        start=(j == 0),
                        stop=(j == TPC - 1),
                    )
                    if j == TPC - 1:
                        rec = work.tile([128, 1], f32, tag="rec")
                        nc.vector.tensor_scalar_add(
                            rec[:], agg_ps[:, F : F + 1], 1.0
                        )
                        nc.vector.reciprocal(rec[:], rec[:])
                        o1 = work.tile([128, F], f32, tag="o1")
                        nc.vector.tensor_tensor(
                            out=o1[:],
                            in0=agg_ps[:, 0:F],
                            in1=rec[:].to_broadcast([128, F]),
                            op=mybir.AluOpType.mult,
                        )
                        o2 = work.tile([128, F], f32, tag="o2")
                        nc.vector.tensor_add(o2[:], o1[:], bias_sb[:])
                        nc.sync.dma_start(
                            out_d[g, W * wd : W * (wd + 1), :], o2[:]
                        )

    nc.compile()
    return nc


def _prep_graph(src, dst):
    """(dst_window, src_window) block sort; returns one-hot packs
    soh, doh [NCHUNK, 128, TPC, 128] float32 (cast to bf16 by caller)."""
    ws = src // W
    wd = dst // W
    blk = wd * NW + ws
    order = np.argsort(blk, kind="stable")
    s, d, bo = src[order], dst[order], blk[order]
    counts = np.bincount(bo, minlength=NW * NW)
    assert counts.max() <= T_B * 128, f"block overflow: {counts.max()}"
    soh = np.zeros((T, 128, 128), dtype=np.float32)  # [tile, node, lane]
    doh = np.zeros((T, 128, 128), dtype=np.float32)  # [tile, lane, node]
    slo = (s % W).astype(np.int64)
    dlo = (d % W).astype(np.int64)
    starts = np.zeros(NW * NW + 1, dtype=np.int64)
    np.cumsum(counts, out=starts[1:])
    pos_in_blk = np.arange(len(s)) - starts[bo]
    tile_idx = bo * T_B + pos_in_blk // 128
    lane = pos_in_blk % 128
    soh[tile_idx, slo, lane] = 1.0
    doh[tile_idx, lane, dlo] = 1.0
    soh = soh.reshape(NCHUNK, TPC, 128, 128).transpose(0, 2, 1, 3).copy()
    doh = doh.reshape(NCHUNK, TPC, 128, 128).transpose(0, 2, 1, 3).copy()
    return soh, doh


def kernel(text, weight, bias, edge_src, edge_dst):
    import ml_dtypes

    text = np.asarray(text, dtype=np.float32)
    weight = np.asarray(weight, dtype=np.float32)
    bias = np.asarray(bias, dtype=np.float32)
    edge_src = np.asarray(edge_src, dtype=np.int32)
    edge_dst = np.asarray(edge_dst, dtype=np.int32)

    if "nc" not in _cache:
        _cache["nc"] = _build_program()
    nc = _cache["nc"]

    bias_rep = np.tile(bias[None, :], (128, 1)).astype(np.float32)

    in_maps = []
    for k in range(NCORES):
        soh = np.empty((GPC, NCHUNK, 128, TPC, 128), dtype=ml_dtypes.bfloat16)
        doh = np.empty((GPC, NCHUNK, 128, TPC, 128), dtype=ml_dtypes.bfloat16)
        for g in range(GPC):
            b = k * GPC + g
            so, do = _prep_graph(edge_src[b], edge_dst[b])
            soh[g] = so.astype(ml_dtypes.bfloat16)
            doh[g] = do.astype(ml_dtypes.bfloat16)
        in_maps.append(
            {
                "text": text[k * GPC : (k + 1) * GPC],
                "weight": weight,
                "biasrep": bias_rep,
                "srcoh": soh,
                "dstoh": doh,
            }
        )

    _cache["in_maps"] = in_maps

    from concourse.bass_utils import run_bass_kernel_spmd

    res = run_bass_kernel_spmd(nc, in_maps, list(range(NCORES)))
    out = np.concatenate([res.results[k]["out"] for k in range(NCORES)], axis=0)
    return out.astype(np.float32)



# revision 2
# speedup vs baseline: 6.3170x; 6.3170x over previous
"""DglGraphConvolution Trainium2 kernel — dense block-adjacency SpMM.

Key idea: segment_sum over edges == A @ x where A[d, s] = multiplicity of
edge (s -> d). Host re-encodes the edge index lists as the dense count
matrix A^T (src-major, fp8 e4m3: counts are tiny ints, exactly
representable) — pure index preprocessing, no model data touched.

Because aggregation and the feature transform are both linear, the device
aggregates RAW text first and applies W after:

  per graph g (2 per core, data-parallel over B=16 on 8 cores):
    stage 1: for ws in 0..31:   (src windows of 128 nodes)
       tagg[fin, d] += text_ws[s, fin]^T-as-stationary @ A^T[s, d]
       (8 matmuls of 512 moving cols -> 8 psum banks; A^T row streamed
        straight from HBM as fp8, text window converted f32->bf16 on ACT;
        stationary operand reused across 8 matmuls -> no LDWEIGHTS wall)
    evict: tagg_n = tagg * recip_deg[d]   (DVE, psum -> bf16 sbuf)
    stage 2: out^T[f, d] = W[fin, f]^T-as-stationary @ tagg_n[fin, d]
    evict: out = out^T + bias[f]          (ACT per-partition bias, -> bf16)

deg comes from the same index-only host prep (recip = 1/(deg+1), f32,
replicated across the 128 partitions). Output is written transposed
[f, d] and untransposed on the host.
"""

import numpy as np

B, N, E, F = 16, 4096, 131072, 128
NCORES = 8
GPC = B // NCORES  # graphs per core
W = 128  # src window (partition) size
NW = N // W  # 32 src windows
Q = 8  # moving-dim quadrants per A row
QW = N // Q  # 512 moving columns per matmul (= MAX_MOVING_FREE_DIM_SIZE)

_cache = {}


def _build_program():
    from contextlib import ExitStack

    import concourse.bacc as bacc
    import concourse.tile as tile
    from concourse import mybir
    from concourse._compat import get_trn_type

    f32 = mybir.dt.float32
    bf16 = mybir.dt.bfloat16
    fp8 = mybir.dt.float8e4

    nc = bacc.Bacc(get_trn_type() or "TRN2", target_bir_lowering=False, debug=False)

    text_d = nc.dram_tensor("text", [GPC, N, F], f32, kind="ExternalInput")
    a_d = nc.dram_tensor("acnt", [GPC, NW, W, N], fp8, kind="ExternalInput")
    rec_d = nc.dram_tensor("recrep", [GPC, W, N], f32, kind="ExternalInput")
    w_d = nc.dram_tensor("weight", [F, F], f32, kind="ExternalInput")
    bias_d = nc.dram_tensor("biascol", [F, 1], f32, kind="ExternalInput")
    out_d = nc.dram_tensor("out", [GPC, F, N], bf16, kind="ExternalOutput")

    with tile.TileContext(nc) as tc, ExitStack() as ctx:
        const = ctx.enter_context(tc.tile_pool(name="const", bufs=1))
        tpool = ctx.enter_context(tc.tile_pool(name="tpool", bufs=3))
        spool = ctx.enter_context(tc.tile_pool(name="spool", bufs=3))
        apool = ctx.enter_context(tc.tile_pool(name="apool", bufs=4))
        gpool = ctx.enter_context(tc.tile_pool(name="gpool", bufs=2))
        opool = ctx.enter_context(tc.tile_pool(name="opool", bufs=3))
        psum = ctx.enter_context(tc.tile_pool(name="psum", bufs=8, space="PSUM"))

        w_sb = const.tile([F, F], f32)
        nc.sync.dma_start(w_sb[:], w_d[:, :])
        w_bf = const.tile([F, F], bf16)
        nc.vector.tensor_copy(w_bf[:], w_sb[:])
        bias_sb = const.tile([F, 1], f32)
        nc.sync.dma_start(bias_sb[:], bias_d[:, :])

        for g in range(GPC):
            recrep = gpool.tile([W, N], f32, tag="rec")
            nc.sync.dma_start(recrep[:], rec_d[g])

            acc = []
            for ws in range(NW):
                tt = tpool.tile([W, F], f32, tag="t")
                nc.sync.dma_start(tt[:], text_d[g, W * ws : W * (ws + 1), :])
                st = spool.tile([W, F], bf16, tag="s")
                nc.scalar.activation(
                    st[:], tt[:], mybir.ActivationFunctionType.Copy
                )
                ar = apool.tile([W, N], fp8, tag="a")
                nc.sync.dma_start(ar[:], a_d[g, ws])
                for q in range(Q):
                    if ws == 0:
                        a_ps = psum.tile([W, QW], f32, tag="acc", name=f"acc{g}_{q}")
                        acc.append(a_ps)
                    nc.tensor.matmul(
                        out=acc[q][:],
                        lhsT=st[:],
                        rhs=ar[:, QW * q : QW * (q + 1)],
                        start=(ws == 0),
                        stop=(ws == NW - 1),
                    )

            tagg = gpool.tile([F, N], bf16, tag="tagg")
            for q in range(Q):
                nc.vector.tensor_tensor(
                    out=tagg[:, QW * q : QW * (q + 1)],
                    in0=acc[q][:],
                    in1=recrep[:, QW * q : QW * (q + 1)],
                    op=mybir.AluOpType.mult,
                )

            for q in range(Q):
                o_ps = psum.tile([F, QW], f32, tag="acc", name=f"ops{g}_{q}")
                nc.tensor.matmul(
                    out=o_ps[:],
                    lhsT=w_bf[:],
                    rhs=tagg[:, QW * q : QW * (q + 1)],
                    start=True,
                    stop=True,
                )
                obf = opool.tile([F, QW], bf16, tag="o")
                nc.scalar.activation(
                    obf[:],
                    o_ps[:],
                    mybir.ActivationFunctionType.Identity,
                    bias=bias_sb[:, 0:1],
                )
                nc.scalar.dma_start(out_d[g, :, QW * q : QW * (q + 1)], obf[:])

    nc.compile()
    return nc


def _prep_graph(src, dst):
    """Index-only: dense src-major count matrix [NW, W, N] (float32 counts)
    and the replicated reciprocal degree row [W, N]."""
    lin = src.astype(np.int64) * N + dst
    cnt = np.bincount(lin, minlength=N * N).astype(np.float32)
    assert cnt.max() <= 16, f"edge multiplicity overflow: {cnt.max()}"
    deg = np.bincount(dst, minlength=N).astype(np.float32)
    rec = (1.0 / (deg + 1.0)).astype(np.float32)
    recrep = np.ascontiguousarray(np.broadcast_to(rec[None, :], (W, N)))
    return cnt.reshape(NW, W, N), recrep


def kernel(text, weight, bias, edge_src, edge_dst):
    import ml_dtypes

    text = np.asarray(text, dtype=np.float32)
    weight = np.asarray(weight, dtype=np.float32)
    bias = np.asarray(bias, dtype=np.float32)
    edge_src = np.asarray(edge_src, dtype=np.int32)
    edge_dst = np.asarray(edge_dst, dtype=np.int32)

    if "nc" not in _cache:
        _cache["nc"] = _build_program()
    nc = _cache["nc"]

    in_maps = []
    for k in range(NCORES):
        acnt = np.empty((GPC, NW, W, N), dtype=ml_dtypes.float8_e4m3)
        recrep = np.empty((GPC, W, N), dtype=np.float32)
        for g in range(GPC):
            b = k * GPC + g
            cnt, rr = _prep_graph(edge_src[b], edge_dst[b])
            acnt[g] = cnt.astype(ml_dtypes.float8_e4m3)
            recrep[g] = rr
        in_maps.append(
            {
                "text": text[k * GPC : (k + 1) * GPC],
                "acnt": acnt,
                "recrep": recrep,
                "weight": weight,
                "biascol": bias.reshape(F, 1),
            }
        )

    _cache["in_maps"] = in_maps

    from concourse.bass_utils import run_bass_kernel_spmd

    res = run_bass_kernel_spmd(nc, in_maps, list(range(NCORES)))
    out = np.concatenate(
        [
            np.asarray(res.results[k]["out"])
            .astype(np.float32)
            .transpose(0, 2, 1)
            for k in range(NCORES)
        ],
        axis=0,
    )
    return out


# revision 7
# speedup vs baseline: 6.3702x; 1.0084x over previous
"""DglGraphConvolution Trainium2 kernel — dense block-adjacency SpMM.

Key idea: segment_sum over edges == A @ x where A[d, s] = multiplicity of
edge (s -> d). Host re-encodes the edge index lists as the dense count
matrix A^T (src-major, fp8 e4m3: counts are tiny ints, exactly
representable) — pure index preprocessing, no model data touched.

Because aggregation and the feature transform are both linear, the device
aggregates RAW text first and applies W after:

  per graph g (2 per core, data-parallel over B=16 on 8 cores):
    stage 1: for ws in 0..31:   (src windows of 128 nodes)
       tagg[fin, d] += text_ws[s, fin]^T-as-stationary @ A^T[s, d]
       (8 matmuls of 512 moving cols -> 8 psum banks; A^T row streamed
        straight from HBM as fp8, text window converted f32->bf16 on ACT;
        stationary operand reused across 8 matmuls -> no LDWEIGHTS wall)
    evict: tagg_n = tagg * recip_deg[d]   (DVE, psum -> bf16 sbuf)
    stage 2: out^T[f, d] = W[fin, f]^T-as-stationary @ tagg_n[fin, d]
    evict: out = out^T + bias[f]          (ACT per-partition bias, -> bf16)

deg comes from the same index-only host prep (recip = 1/(deg+1), f32,
replicated across the 128 partitions). Output is written transposed
[f, d] and untransposed on the host.
"""

import numpy as np

B, N, E, F = 16, 4096, 131072, 128
NCORES = 8
GPC = B // NCORES  # graphs per core
W = 128  # src window (partition) size
NW = N // W  # 32 src windows
Q = 8  # moving-dim quadrants per A row
QW = N // Q  # 512 moving columns per matmul (= MAX_MOVING_FREE_DIM_SIZE)

_cache = {}


def _build_program():
    from contextlib import ExitStack

    import concourse.bacc as bacc
    import concourse.tile as tile
    from concourse import mybir
    from concourse._compat import get_trn_type

    f32 = mybir.dt.float32
    bf16 = mybir.dt.bfloat16
    fp8 = mybir.dt.float8e4

    nc = bacc.Bacc(get_trn_type() or "TRN2", target_bir_lowering=False, debug=False)

    text_d = nc.dram_tensor("text", [GPC, N, F], f32, kind="ExternalInput")
    a_d = nc.dram_tensor("acnt", [GPC, NW, W, N], fp8, kind="ExternalInput")
    rec_d = nc.dram_tensor("recrep", [GPC, W, N], bf16, kind="ExternalInput")
    w_d = nc.dram_tensor("weight", [F, F], f32, kind="ExternalInput")
    bias_d = nc.dram_tensor("biascol", [F, 1], f32, kind="ExternalInput")
    out_d = nc.dram_tensor("out", [GPC, F, N], bf16, kind="ExternalOutput")

    with tile.TileContext(nc) as tc, ExitStack() as ctx:
        const = ctx.enter_context(tc.tile_pool(name="const", bufs=1))
        tpool = ctx.enter_context(tc.tile_pool(name="tpool", bufs=3))
        spool = ctx.enter_context(tc.tile_pool(name="spool", bufs=3))
        apool = ctx.enter_context(tc.tile_pool(name="apool", bufs=6))
        gpool = ctx.enter_context(tc.tile_pool(name="gpool", bufs=2))
        opool = ctx.enter_context(tc.tile_pool(name="opool", bufs=3))
        psum = ctx.enter_context(tc.tile_pool(name="psum", bufs=8, space="PSUM"))

        w_sb = const.tile([F, F], f32)
        nc.sync.dma_start(w_sb[:], w_d[:, :])
        w_bf = const.tile([F, F], bf16)
        nc.vector.tensor_copy(w_bf[:], w_sb[:])
        bias_sb = const.tile([F, 1], f32)
        nc.sync.dma_start(bias_sb[:], bias_d[:, :])

        for g in range(GPC):
            recrep = gpool.tile([W, N], bf16, tag="rec")

            acc = []
            for ws in range(NW):
                tt = tpool.tile([W, F], f32, tag="t")
                nc.sync.dma_start(tt[:], text_d[g, W * ws : W * (ws + 1), :])
                st = spool.tile([W, F], bf16, tag="s")
                nc.scalar.activation(
                    st[:], tt[:], mybir.ActivationFunctionType.Copy
                )
                ar = apool.tile([W, N], fp8, tag="a")
                nc.sync.dma_start(ar[:], a_d[g, ws])
                if ws == 8:
                    # deferred: not needed until psum eviction, keep the
                    # head of the stream free for the first A rows
                    nc.sync.dma_start(recrep[:], rec_d[g])
                for q in range(Q):
                    if ws == 0:
                        a_ps = psum.tile([W, QW], f32, tag="acc", name=f"acc{g}_{q}")
                        acc.append(a_ps)
                    nc.tensor.matmul(
                        out=acc[q][:],
                        lhsT=st[:],
                        rhs=ar[:, QW * q : QW * (q + 1)],
                        start=(ws == 0),
                        stop=(ws == NW - 1),
                    )

            tagg = gpool.tile([F, N], bf16, tag="tagg")
            for q in range(Q):
                nc.vector.tensor_tensor(
                    out=tagg[:, QW * q : QW * (q + 1)],
                    in0=acc[q][:],
                    in1=recrep[:, QW * q : QW * (q + 1)],
                    op=mybir.AluOpType.mult,
                )

            for q in range(Q):
                o_ps = psum.tile([F, QW], f32, tag="acc", name=f"ops{g}_{q}")
                nc.tensor.matmul(
                    out=o_ps[:],
                    lhsT=w_bf[:],
                    rhs=tagg[:, QW * q : QW * (q + 1)],
                    start=True,
                    stop=True,
                )
                obf = opool.tile([F, QW], bf16, tag="o")
                nc.scalar.activation(
                    obf[:],
                    o_ps[:],
                    mybir.ActivationFunctionType.Identity,
                    bias=bias_sb[:, 0:1],
                )
                nc.sync.dma_start(out_d[g, :, QW * q : QW * (q + 1)], obf[:])

    nc.compile()
    return nc


def _prep_graph(src, dst):
    """Index-only: dense src-major count matrix [NW, W, N] (float32 counts)
    and the replicated reciprocal degree row [W, N]."""
    lin = src.astype(np.int64) * N + dst
    cnt = np.bincount(lin, minlength=N * N).astype(np.float32)
    assert cnt.max() <= 16, f"edge multiplicity overflow: {cnt.max()}"
    deg = np.bincount(dst, minlength=N).astype(np.float32)
    rec = (1.0 / (deg + 1.0)).astype(np.float32)
    recrep = np.ascontiguousarray(np.broadcast_to(rec[None, :], (W, N)))
    return cnt.reshape(NW, W, N), recrep


def kernel(text, weight, bias, edge_src, edge_dst):
    import ml_dtypes

    text = np.asarray(text, dtype=np.float32)
    weight = np.asarray(weight, dtype=np.float32)
    bias = np.asarray(bias, dtype=np.float32)
    edge_src = np.asarray(edge_src, dtype=np.int32)
    edge_dst = np.asarray(edge_dst, dtype=np.int32)

    if "nc" not in _cache:
        _cache["nc"] = _build_program()
    nc = _cache["nc"]

    in_maps = []
    for k in range(NCORES):
        acnt = np.empty((GPC, NW, W, N), dtype=ml_dtypes.float8_e4m3)
        recrep = np.empty((GPC, W, N), dtype=ml_dtypes.bfloat16)
        for g in range(GPC):
            b = k * GPC + g
            cnt, rr = _prep_graph(edge_src[b], edge_dst[b])
            acnt[g] = cnt.astype(ml_dtypes.float8_e4m3)
            recrep[g] = rr
        in_maps.append(
            {
                "text": text[k * GPC : (k + 1) * GPC],
                "acnt": acnt,
                "recrep": recrep,
                "weight": weight,
                "biascol": bias.reshape(F, 1),
            }
        )

    _cache["in_maps"] = in_maps

    from concourse.bass_utils import run_bass_kernel_spmd

    res = run_bass_kernel_spmd(nc, in_maps, list(range(NCORES)))
    out = np.concatenate(
        [
            np.asarray(res.results[k]["out"])
            .astype(np.float32)
            .transpose(0, 2, 1)
            for k in range(NCORES)
        ],
        axis=0,
    )
    return out
